# revision 17
# baseline (speedup 1.0000x reference)
"""Trainium2 Bass kernel for nn_MixtureOfExperts (argmax-routed SwiGLU MoE).

Strategy (expert-parallel across 8 NeuronCores):
  - Host computes router logits (fp64 matmul, tiny: 4096x1024x8) and the
    argmax expert per token.  Top-2 logit gaps are >=1.7e-4 while fp32
    rounding noise is ~1e-6, so routing is insensitive to arithmetic order.
  - Tokens are grouped by selected expert; each core receives only its
    expert's tokens (padded to a common capacity C) plus that expert's
    gate/up/down banks.  Each core computes the SwiGLU for its tokens only:
        h = silu(x @ gw) * (x @ uw);  y = h @ dw
    This does 1/E of the reference FLOPs (the reference computes all E
    experts densely and discards all but the argmax one).
  - Host scatters per-core outputs back to token positions.

Layout choices: x is shipped pre-transposed as [D, C] so the contraction
dim D lands on SBUF partitions for both matmul operands; mm1 produces
h^T [H, C] tiles which are exactly the stationary operand layout needed
for mm2 (contraction over H), so no on-chip transposes are required.
gate/up weights are host-packed into one array so each weight DMA is a
single large contiguous transfer.

Matmul dtype: float32r (fp32 storage, PE rounds to 11 mantissa bits,
1 cycle/row vs fp32's 4).  Operands are pre-rounded on the host with the
exact RNE-to-12-low-bits rule of the compiler's fp32_to_fp32r, so DMA'd
data is already on the f32r grid.
"""

import numpy as np

import concourse.bass as bass  # noqa: F401  (kept for API users)
import concourse.mybir as mybir
import concourse.tile as tile
from concourse import bacc
from concourse.bass_utils import run_bass_kernel_spmd

B, T, D, E, H = 4, 1024, 1024, 8, 2048
BT = B * T
NCORES = 8
P = 128
F32 = mybir.dt.float32
F32R = mybir.dt.float32r

# "fp32"  : exact fp32 matmuls (4 cycles/row on PE)
# "fp32r" : fp32 data, reduced-precision PE mode (1 cycle/row at N>=256)
MM_MODE = "fp32r"

# gate/up weight-fetch chunks over H (sum = H).  The first chunks are small
# so the critical prefix (x + first weight chunk) is minimal before the PE
# can start.
H_CHUNKS = [(0, 128), (128, 128), (256, 256), (512, 512), (1024, 512), (1536, 512)]

_BUILD_CACHE = {}


def _chunks(total, step):
    out = []
    o = 0
    while o < total:
        out.append((o, min(step, total - o)))
        o += step
    return out


def _balanced_chunks(total, step):
    """Split `total` into chunks <= step, as equal as possible (32-aligned).

    Keeps every matmul free-dim >= 256 where possible, which float32r needs
    for full-rate (1 cycle/row) operation.
    """
    n = -(-total // step)
    base = -(-total // (n * 32)) * 32
    out = []
    o = 0
    while o < total:
        sz = min(base, total - o)
        out.append((o, sz))
        o += sz
    return out


def round_fp32r(a):
    """Round fp32 array to the float32r grid (RNE at 12 low mantissa bits).

    Bit-exact with neuronxcc's fp32_to_fp32r (verified exhaustively on
    random normals + subnormal/huge ranges).
    """
    u = np.ascontiguousarray(a, np.float32).view(np.uint32).astype(np.uint64)
    lsb = (u >> 12) & 1
    r = (u + 0x7FF + lsb) & 0xFFFFF000
    return r.astype(np.uint32).view(np.float32)


def _build(C, mm_mode):
    """Build the per-core SPMD Bass kernel for token capacity C."""
    KD = D // P   # k-tiles for mm1 (contraction over D)
    KH = H // P   # k-tiles for mm2 (contraction over H)
    n_chunks = _balanced_chunks(C, 512)   # token tiles in the free dim (mm1)
    m_tiles = _chunks(C, P)               # token tiles in the partition dim (mm2)
    d_chunks = _chunks(D, 512)            # output-column tiles (mm2)
    HC = H_CHUNKS

    mdt = F32R if mm_mode == "fp32r" else F32

    nc = bacc.Bacc("TRN2", target_bir_lowering=False, debug=False)
    xt = nc.dram_tensor("xt", [D, C], mdt, kind="ExternalInput")
    gu = nc.dram_tensor("gu", [D, 2 * H], mdt, kind="ExternalInput")
    dw = nc.dram_tensor("dw", [H, D], mdt, kind="ExternalInput")
    y = nc.dram_tensor("y", [C, D], F32, kind="ExternalOutput")

    with tile.TileContext(nc) as tc:
        with (
            tc.tile_pool(name="xp", bufs=KD) as xp,
            tc.tile_pool(name="hp", bufs=KH) as hp,
            tc.tile_pool(name="w1", bufs=16) as w1,
            tc.tile_pool(name="w2", bufs=KH) as w2,
            tc.tile_pool(name="outp", bufs=3) as outp,
            tc.tile_pool(name="ps1", bufs=6, space="PSUM") as ps1,
            tc.tile_pool(name="ps2", bufs=2, space="PSUM") as ps2,
        ):
            # resident activations: x^T k-tiles, one per (k, token-chunk) so
            # the first compute group only waits on the first token chunk.
            # Later token chunks are DMA'd after the first weight chunk (see
            # below) to keep the critical prefix minimal.
            x_t = {}

            def load_x_chunk(ni):
                n0, nn_ = n_chunks[ni]
                for k in range(KD):
                    t = xp.tile([P, nn_], mdt, tag=f"x{ni}")
                    nc.sync.dma_start(t[:], xt[k * P:(k + 1) * P, n0:n0 + nn_])
                    x_t[k, ni] = t

            load_x_chunk(0)

            # ---- mm1: hT[j] = silu(gw.T x) * (uw.T x), tiled over H ----
            h_t = []
            for ci, (hc0, hcn) in enumerate(HC):
                # packed [gate chunk | up chunk] per k-slice: one DMA each
                gut = []
                for k in range(KD):
                    t = w1.tile([P, 2 * hcn], mdt, tag="w1")
                    nc.sync.dma_start(
                        t[:], gu[k * P:(k + 1) * P,
                                 2 * hc0:2 * hc0 + 2 * hcn])
                    gut.append(t)
                if ci == 0:
                    for ni in range(1, len(n_chunks)):
                        load_x_chunk(ni)
                for hs in range(hcn // P):
                    ht = hp.tile([P, C], mdt, tag="h")
                    for ni, (n0, nn_) in enumerate(n_chunks):
                        pa = ps1.tile([P, nn_], F32, tag="ps1")
                        pu = ps1.tile([P, nn_], F32, tag="ps1")
                        for k in range(KD):
                            nc.tensor.matmul(
                                pa[:, :],
                                lhsT=gut[k][:, hs * P:(hs + 1) * P],
                                rhs=x_t[k, ni][:, :],
                                start=(k == 0),
                                stop=(k == KD - 1),
                            )
                        for k in range(KD):
                            nc.tensor.matmul(
                                pu[:, :],
                                lhsT=gut[k][:, hcn + hs * P:hcn + (hs + 1) * P],
                                rhs=x_t[k, ni][:, :],
                                start=(k == 0),
                                stop=(k == KD - 1),
                            )
                        nc.scalar.activation(
                            ht[:, n0:n0 + nn_], pa[:, :],
                            mybir.ActivationFunctionType.Silu,
                        )
                        nc.vector.tensor_mul(
                            ht[:, n0:n0 + nn_], ht[:, n0:n0 + nn_], pu[:, :]
                        )
                    h_t.append(ht)

            # down-proj weights: one [128, D] tile per h k-slice, loaded
            # once and reused by every (nd, m) tile.  Emitted after mm1 so
            # the DMA queue drains mm1's weights first; the scheduler still
            # overlaps these loads with mm1 compute.
            dwt = []
            for k in range(KH):
                t = w2.tile([P, D], mdt, tag="w2")
                nc.sync.dma_start(t[:], dw[k * P:(k + 1) * P, :])
                dwt.append(t)

            # ---- mm2: y = h @ dw, contraction over H ----
            for nd0, ndn in d_chunks:
                for m0, mn in m_tiles:
                    py = ps2.tile([P, ndn], F32, tag="ps2")
                    for k in range(KH):
                        nc.tensor.matmul(
                            py[:mn, :],
                            lhsT=h_t[k][:, m0:m0 + mn],
                            rhs=dwt[k][:, nd0:nd0 + ndn],
                            start=(k == 0),
                            stop=(k == KH - 1),
                        )
                    ot = outp.tile([P, ndn], F32, tag="out")
                    nc.vector.tensor_copy(ot[:mn, :], py[:mn, :])
                    nc.sync.dma_start(y[m0:m0 + mn, nd0:nd0 + ndn], ot[:mn, :])

    nc.compile()
    return nc


def _get_kernel(C, mm_mode=None):
    mm_mode = mm_mode or MM_MODE
    key = (C, mm_mode)
    if key not in _BUILD_CACHE:
        _BUILD_CACHE[key] = _build(C, mm_mode)
    return _BUILD_CACHE[key]


def _route(xf, gate_w):
    """argmax expert per token, computed in fp64 on host (negligible work)."""
    logits = xf.astype(np.float64) @ np.asarray(gate_w, np.float64).T
    return logits.argmax(axis=1)


def _pack_gu(gw_e, uw_e):
    """Interleave gate/up banks by H_CHUNKS columns: [D, 2H] with chunk i at
    [:, 2*hc0 : 2*(hc0+hcn)] = [gate chunk | up chunk]."""
    parts = []
    for hc0, hcn in H_CHUNKS:
        parts.append(gw_e[:, hc0:hc0 + hcn])
        parts.append(uw_e[:, hc0:hc0 + hcn])
    return np.ascontiguousarray(np.concatenate(parts, axis=1))


def kernel(x, gate_w, gate_bank, up_bank, down_bank):
    x = np.asarray(x, np.float32)
    assert x.shape == (B, T, D)

    xf = np.ascontiguousarray(x.reshape(BT, D))
    sel = _route(xf, gate_w)
    idx = [np.nonzero(sel == e)[0] for e in range(E)]
    maxc = max(len(i) for i in idx)
    C = max(P, -(-maxc // 32) * 32)

    nc = _get_kernel(C)

    rnd = round_fp32r if MM_MODE == "fp32r" else (
        lambda a: np.ascontiguousarray(a, np.float32))
    gate_bank = rnd(gate_bank)
    up_bank = rnd(up_bank)
    down_bank = rnd(down_bank)

    in_maps = []
    for e in range(E):
        xe = np.zeros((D, C), np.float32)
        n = len(idx[e])
        if n:
            xe[:, :n] = rnd(xf[idx[e]].T)
        in_maps.append({
            "xt": xe,
            "gu": _pack_gu(gate_bank[e], up_bank[e]),
            "dw": np.ascontiguousarray(down_bank[e]),
        })

    res = run_bass_kernel_spmd(nc, in_maps, core_ids=list(range(NCORES)))

    out = np.empty((BT, D), np.float32)
    for e in range(E):
        n = len(idx[e])
        if n:
            out[idx[e]] = res.results[e]["y"][:n]
    return out.reshape(B, T, D)
